# revision 1
# baseline (speedup 1.0000x reference)
"""CLIP contrastive loss on 8 Trainium2 NeuronCores (Bass/Tile).

Strategy (data-parallel over image rows, hint's local_loss path):
  - Core c holds image rows [c*1024, (c+1)*1024) and the FULL text matrix.
  - Text rows are rolled by c*1024 on the host so every core's diagonal
    block sits at local column 0 (the compiled program is core-independent).
  - On device, each core computes its 1024 x 8192 logits block in
    128x1024 wide PSUM tiles (two 512-col halves, 4 accumulating K=128
    bf16 matmuls each), then:
      * ACT: one exp(scale*s - shift) per wide tile PSUM->SBUF (bf16),
        accum_out = per-row sums (free with the exp pass)
      * DVE: adds exp tiles into a per-nt [128,1024] bf16 column
        accumulator (DMA'd out whole); per-mt diagonal extracted with
        tensor_mul against scale*I + reduce
  - Host: partition-reduces the column accumulators and combines
    per-core row/col exp-sums and diagonals in float64:
      lse = shift + log(sum); loss = mean over both directions.

Fixed-shift logsumexp is numerically safe: logits = scale*cos(theta) are
bounded by +-scale, and shift = scale/2 keeps every term that matters in
normal f32 range (terms below exp(-87) are negligible vs the row max).
"""

from contextlib import ExitStack

import numpy as np

import concourse.bass as bass
from concourse import bacc
import concourse.tile as tile
from concourse import mybir
from concourse.bass import ts
from concourse.bass_utils import run_bass_kernel_spmd

N = 8192
D = 512
NC = 8
M_LOC = N // NC          # 1024 image rows per core
MT = M_LOC // 128        # 8 m-tiles of 128 rows
NT = N // 512            # 16 n-tiles of 512 text cols
KC = D // 128            # 4 contraction chunks

F32 = mybir.dt.float32
BF16 = mybir.dt.bfloat16

# Matmul input dtype: "f32" (exact) or "bf16" (4x PE throughput, ~1e-5 loss err)
MM_DTYPE = "bf16"
# Single matmul streaming 1024 bf16 columns (2 PSUM banks) vs two 512-col MMs
WIDE_MM = False

_CACHE = {}
LAST_RESULTS = None


def _build(scale: float, shift: float, mm_dtype: str, dims=None):
    n, m_loc, kc_n = (N, M_LOC, KC) if dims is None else dims
    mt_n, nt_n = m_loc // 128, n // 1024
    mmdt = F32 if mm_dtype == "f32" else BF16
    nc = bacc.Bacc("TRN2", debug=False)

    at_d = nc.dram_tensor("at_in", [128, kc_n, m_loc], mmdt, kind="ExternalInput").ap()
    bt_d = nc.dram_tensor("bt_in", [nt_n, 128, kc_n, 1024], mmdt, kind="ExternalInput").ap()
    eye_d = nc.dram_tensor("eye_in", [128, 128], F32, kind="ExternalInput").ap()

    rowsum_d = nc.dram_tensor("rowsum_out", [128, mt_n], F32, kind="ExternalOutput").ap()
    colsum_d = nc.dram_tensor("colsum_out", [nt_n, 128, 1024], mmdt, kind="ExternalOutput").ap()
    diag_d = nc.dram_tensor("diag_out", [128, mt_n], F32, kind="ExternalOutput").ap()

    with ExitStack() as ctx:
        tc = ctx.enter_context(tile.TileContext(nc))
        singles = ctx.enter_context(tc.tile_pool(name="singles", bufs=1))
        btp = ctx.enter_context(tc.tile_pool(name="btp", bufs=nt_n))
        expp = ctx.enter_context(tc.tile_pool(name="expp", bufs=10))
        scr = ctx.enter_context(tc.tile_pool(name="scr", bufs=3))
        psum = ctx.enter_context(tc.tile_pool(name="psum", bufs=4, space="PSUM"))

        at_t = singles.tile([128, kc_n, m_loc], mmdt)
        bt_tiles = [
            btp.tile([128, kc_n, 1024], mmdt, name=f"bt{nt}", tag="bt")
            for nt in range(nt_n)
        ]
        # Per-chunk loads for the first tiles so the first matmul group can
        # start as soon as its (at, bt0) K-chunks land, not after 2 MB.
        for kc in range(kc_n):
            nc.sync.dma_start(at_t[:, kc, :], at_d[:, kc, :])
            nc.sync.dma_start(bt_tiles[0][:, kc, :], bt_d[0, :, kc, :])
        eye_t = singles.tile([128, 128], F32)
        nc.sync.dma_start(eye_t, eye_d)
        bias_t = singles.tile([128, 1], F32)
        nc.vector.memset(bias_t, -shift)

        rowpart = singles.tile([128, mt_n, nt_n], F32)
        rowsum_sb = singles.tile([128, mt_n], F32)
        diag_sb = singles.tile([128, mt_n], F32)

        for nt in range(1, nt_n):
            nc.sync.dma_start(bt_tiles[nt], bt_d[nt])

        for nt in range(nt_n):
            colacc_sb = scr.tile([128, 1024], mmdt, name=f"cacc{nt}", tag="caccsb", bufs=3)
            for mt in range(mt_n):
                s_ps = psum.tile([128, 1024], F32, name=f"s{nt}_{mt}", tag="spsum")
                for kc in range(kc_n):
                    if WIDE_MM:
                        nc.tensor.matmul(
                            s_ps,
                            at_t[:, kc, ts(mt, 128)],
                            bt_tiles[nt][:, kc, :],
                            start=(kc == 0),
                            stop=(kc == kc_n - 1),
                        )
                    else:
                        for h in range(2):
                            nc.tensor.matmul(
                                s_ps[:, ts(h, 512)],
                                at_t[:, kc, ts(mt, 128)],
                                bt_tiles[nt][:, kc, ts(h, 512)],
                                start=(kc == 0),
                                stop=(kc == kc_n - 1),
                            )
                if nt == (mt * 128) // 1024:
                    # this tile holds the local diagonal block for mt
                    o = (mt * 128) % 1024
                    dscr = scr.tile([128, 128], F32, name=f"dscr{mt}", tag="dscr")
                    nc.vector.tensor_mul(dscr, s_ps[:, o : o + 128], eye_t)
                    nc.vector.tensor_reduce(
                        out=diag_sb[:, mt : mt + 1],
                        in_=dscr,
                        axis=mybir.AxisListType.X,
                        op=mybir.AluOpType.add,
                    )
                e_t = expp.tile([128, 1024], mmdt, name=f"e{nt}_{mt}", tag="exp")
                nc.scalar.activation(
                    e_t,
                    s_ps,
                    mybir.ActivationFunctionType.Exp,
                    bias=bias_t,
                    scale=scale,
                    accum_out=rowpart[:, mt, nt : nt + 1],
                )
                if mt == 0:
                    nc.vector.tensor_copy(colacc_sb, e_t)
                else:
                    nc.vector.tensor_add(colacc_sb, colacc_sb, e_t)
            nc.sync.dma_start(colsum_d[nt], colacc_sb)

        for mt in range(mt_n):
            nc.vector.tensor_reduce(
                out=rowsum_sb[:, mt : mt + 1],
                in_=rowpart[:, mt, :],
                axis=mybir.AxisListType.X,
                op=mybir.AluOpType.add,
            )
        nc.sync.dma_start(rowsum_d, rowsum_sb)
        nc.sync.dma_start(diag_d, diag_sb)

    nc.compile()
    return nc


def _prep_inputs(img, txt, scale, mm_dtype):
    np_mmdt = np.float32 if mm_dtype == "f32" else np.dtype("bfloat16")
    try:
        np.dtype(np_mmdt)
    except TypeError:  # numpy without native bf16: use ml_dtypes
        pass
    if mm_dtype != "f32":
        import ml_dtypes

        np_mmdt = ml_dtypes.bfloat16

    eye = (scale * np.eye(128)).astype(np.float32)
    in_maps = []
    for c in range(NC):
        A = img[c * M_LOC : (c + 1) * M_LOC]                    # [1024, 512]
        at = np.ascontiguousarray(
            A.T.reshape(KC, 128, M_LOC).transpose(1, 0, 2)
        ).astype(np_mmdt)                                       # [128, 4, 1024]
        tr = np.roll(txt, -c * M_LOC, axis=0)                   # local col j -> global (j + c*1024) % N
        bt = np.ascontiguousarray(
            tr.T.reshape(KC, 128, N // 1024, 1024).transpose(2, 1, 0, 3)
        ).astype(np_mmdt)                                       # [8, 128, 4, 1024]
        in_maps.append({"at_in": at, "bt_in": bt, "eye_in": eye})
    return in_maps


def kernel(image_features, text_features, logit_scale):
    global LAST_RESULTS
    img = np.ascontiguousarray(np.asarray(image_features, dtype=np.float32))
    txt = np.ascontiguousarray(np.asarray(text_features, dtype=np.float32))
    scale = float(np.asarray(logit_scale))
    shift = 0.5 * scale

    key = (scale, MM_DTYPE)
    if key not in _CACHE:
        _CACHE[key] = _build(scale, shift, MM_DTYPE)
    nc = _CACHE[key]

    in_maps = _prep_inputs(img, txt, scale, MM_DTYPE)
    res = run_bass_kernel_spmd(nc, in_maps, core_ids=list(range(NC)))
    LAST_RESULTS = res

    colsum_tot = np.zeros(N, dtype=np.float64)
    lse_rows = []
    diags = []
    for c, r in enumerate(res.results):
        rowsum = r["rowsum_out"].astype(np.float64)             # [128, MT] @ [p, mt]
        lse_rows.append(shift + np.log(rowsum.T.reshape(-1)))   # row = mt*128 + p
        diags.append(r["diag_out"].astype(np.float64).T.reshape(-1))
        colsum_tot += np.roll(
            r["colsum_out"].astype(np.float64).sum(axis=1).reshape(-1), c * M_LOC
        )
    lse_row = np.concatenate(lse_rows)
    diag = np.concatenate(diags)
    lse_col = shift + np.log(colsum_tot)

    loss = 0.5 * (np.mean(lse_row - diag) + np.mean(lse_col - diag))
    return np.float32(loss)



# revision 2
# speedup vs baseline: 1.6896x; 1.6896x over previous
"""CLIP contrastive loss on 8 Trainium2 NeuronCores (Bass/Tile), fp8 DoubleRow.

Strategy (data-parallel over image rows, hint's local_loss path):
  - Core c holds image rows [c*1024, (c+1)*1024) and the FULL text matrix.
  - Text rows are rolled by c*1024 on the host so every core's diagonal
    block sits at local cols [0, 1024) (the compiled program is
    core-independent).
  - Features are scaled by 16 on the host and quantized to fp8 e4m3; the
    PE runs DoubleRow matmuls (2 fp8 k-planes per cell, K=256 per MM) at
    ~2x bf16 throughput.  PSUM accumulates exact f32; logits = psum *
    (scale/256).
  - Loop: for each 2048-col group (4 of them), for each 128-row m-tile
    (8): 2 kc x (2048/MM_W) DoubleRow MMs -> [128, 2048] PSUM (4 banks),
    then ONE ACT exp over the whole span (bf16 out, accum_out = partial
    row sums), and a DVE add into the group's column accumulator.
  - Diagonals all live in group 0 (cols mt*128..mt*128+128): DVE
    tensor_mul with (scale/256)*I + free-axis reduce.
  - Host: partition-reduce column accumulators, combine row/col exp-sums
    and diagonals in float64: lse = shift + log(sum); mean over both
    directions.

Fixed-shift logsumexp is numerically safe: logits are bounded by +-scale
and shift = scale/2 keeps every term that matters in normal f32 range.
"""

from contextlib import ExitStack

import numpy as np
import ml_dtypes

import concourse.bass as bass
from concourse import bacc
import concourse.tile as tile
from concourse import mybir
from concourse.bass import ts
from concourse.bass_utils import run_bass_kernel_spmd

N = 8192
D = 512
NC = 8
M_LOC = N // NC          # 1024 image rows per core
MT = M_LOC // 128        # 8 m-tiles of 128 rows
NG = 4                   # column groups
GW = N // NG             # 2048 cols per group
KC = 2                   # DoubleRow K-chunks (256 each)
PRE = 16.0               # host-side fp8 pre-scale per operand

F32 = mybir.dt.float32
BF16 = mybir.dt.bfloat16
FP8 = mybir.dt.float8e4

# moving-free per matmul: out width = MM_W, moving fp8 elements = 2*MM_W
MM_W = 512

_CACHE = {}
LAST_RESULTS = None


def _build(scale: float, shift: float, mm_w: int):
    n_mm = GW // mm_w            # matmuls per (kc, group-span)
    act_scale = scale / (PRE * PRE)
    nc = bacc.Bacc("TRN2", debug=False)

    at_d = nc.dram_tensor("at_in", [128, KC, 2, M_LOC], FP8, kind="ExternalInput").ap()
    bt_d = nc.dram_tensor("bt_in", [NG, KC, 128, 2, GW], FP8, kind="ExternalInput").ap()
    eye_d = nc.dram_tensor("eye_in", [128, 128], F32, kind="ExternalInput").ap()

    rowpart_d = nc.dram_tensor("rowpart_out", [128, MT, NG], F32, kind="ExternalOutput").ap()
    colsum_d = nc.dram_tensor("colsum_out", [NG, 128, GW], BF16, kind="ExternalOutput").ap()
    diag_d = nc.dram_tensor("diag_out", [128, MT], F32, kind="ExternalOutput").ap()

    with ExitStack() as ctx:
        tc = ctx.enter_context(tile.TileContext(nc))
        singles = ctx.enter_context(tc.tile_pool(name="singles", bufs=1))
        btp = ctx.enter_context(tc.tile_pool(name="btp", bufs=NG * KC))
        expp = ctx.enter_context(tc.tile_pool(name="expp", bufs=3))
        cap = ctx.enter_context(tc.tile_pool(name="cap", bufs=2))
        scr = ctx.enter_context(tc.tile_pool(name="scr", bufs=2))
        psum = ctx.enter_context(tc.tile_pool(name="psum", bufs=2, space="PSUM"))

        at_t = singles.tile([128, KC, 2, M_LOC], FP8)
        bt_tiles = [
            [btp.tile([128, 2, GW], FP8, name=f"bt{g}_{kc}", tag="bt") for kc in range(KC)]
            for g in range(NG)
        ]
        # Load order: image (needed by every MM), then text groups in use
        # order so group 0's first matmuls can start early.
        for kc in range(KC):
            nc.sync.dma_start(at_t[:, kc], at_d[:, kc])
        eye_t = singles.tile([128, 128], F32)
        nc.sync.dma_start(eye_t, eye_d)
        bias_t = singles.tile([128, 1], F32)
        nc.vector.memset(bias_t, -shift)

        rowpart = singles.tile([128, MT, NG], F32)
        diag_sb = singles.tile([128, MT], F32)

        for g in range(NG):
            for kc in range(KC):
                nc.sync.dma_start(bt_tiles[g][kc], bt_d[g, kc])

        for g in range(NG):
            colacc = cap.tile([128, GW], BF16, name=f"cacc{g}", tag="cacc")
            for mt in range(MT):
                s_ps = psum.tile([128, GW], F32, name=f"s{g}_{mt}", tag="spsum")
                for kc in range(KC):
                    lhsT = at_t[:, kc, :, ts(mt, 128)]          # [128, 2, 128]
                    for w in range(n_mm):
                        nc.tensor.matmul(
                            s_ps[:, ts(w, mm_w)],
                            lhsT,
                            bt_tiles[g][kc][:, :, ts(w, mm_w)],  # [128, 2, mm_w]
                            start=(kc == 0),
                            stop=(kc == KC - 1),
                            perf_mode=mybir.MatmulPerfMode.DoubleRow,
                        )
                if g == 0:
                    # diag block for mt sits at local cols [mt*128, mt*128+128)
                    dscr = scr.tile([128, 128], F32, name=f"dscr{mt}", tag="dscr")
                    nc.vector.tensor_mul(dscr, s_ps[:, ts(mt, 128)], eye_t)
                    nc.vector.tensor_reduce(
                        out=diag_sb[:, mt : mt + 1],
                        in_=dscr,
                        axis=mybir.AxisListType.X,
                        op=mybir.AluOpType.add,
                    )
                e_t = expp.tile([128, GW], BF16, name=f"e{g}_{mt}", tag="exp")
                nc.scalar.activation(
                    e_t,
                    s_ps,
                    mybir.ActivationFunctionType.Exp,
                    bias=bias_t,
                    scale=act_scale,
                    accum_out=rowpart[:, mt, g : g + 1],
                )
                if mt == 0:
                    nc.vector.tensor_copy(colacc, e_t)
                else:
                    nc.vector.tensor_add(colacc, colacc, e_t)
            nc.sync.dma_start(colsum_d[g], colacc)

        nc.sync.dma_start(rowpart_d, rowpart)
        nc.sync.dma_start(diag_d, diag_sb)

    nc.compile()
    return nc


def _prep_inputs(img, txt, scale):
    fp8 = ml_dtypes.float8_e4m3fn
    eye = ((scale / (PRE * PRE)) * np.eye(128)).astype(np.float32)
    in_maps = []
    for c in range(NC):
        A = (PRE * img[c * M_LOC : (c + 1) * M_LOC]).astype(fp8)   # [1024, 512]
        # k = kc*256 + ko*128 + p
        at = np.ascontiguousarray(
            A.T.reshape(KC, 2, 128, M_LOC).transpose(2, 0, 1, 3)
        )                                                          # [128, KC, 2, 1024]
        tr = np.roll(txt, -c * M_LOC, axis=0)                      # local col j -> global (j + c*1024) % N
        B = (PRE * tr).astype(fp8)                                 # [8192, 512]
        bt = np.ascontiguousarray(
            B.T.reshape(KC, 2, 128, NG, GW).transpose(3, 0, 2, 1, 4)
        )                                                          # [NG, KC, 128, 2, GW]
        in_maps.append({"at_in": at, "bt_in": bt, "eye_in": eye})
    return in_maps


def kernel(image_features, text_features, logit_scale):
    global LAST_RESULTS
    img = np.ascontiguousarray(np.asarray(image_features, dtype=np.float32))
    txt = np.ascontiguousarray(np.asarray(text_features, dtype=np.float32))
    scale = float(np.asarray(logit_scale))
    shift = 0.5 * scale

    key = (scale, MM_W)
    if key not in _CACHE:
        _CACHE[key] = _build(scale, shift, MM_W)
    nc = _CACHE[key]

    in_maps = _prep_inputs(img, txt, scale)
    res = run_bass_kernel_spmd(nc, in_maps, core_ids=list(range(NC)))
    LAST_RESULTS = res

    colsum_tot = np.zeros(N, dtype=np.float64)
    lse_rows = []
    diags = []
    for c, r in enumerate(res.results):
        rowsum = r["rowpart_out"].astype(np.float64).sum(axis=2)    # [128, MT]
        lse_rows.append(shift + np.log(rowsum.T.reshape(-1)))       # row = mt*128 + p
        diags.append(r["diag_out"].astype(np.float64).T.reshape(-1))
        colsum_tot += np.roll(
            r["colsum_out"].astype(np.float64).sum(axis=1).reshape(-1), c * M_LOC
        )
    lse_row = np.concatenate(lse_rows)
    diag = np.concatenate(diags)
    lse_col = shift + np.log(colsum_tot)

    loss = 0.5 * (np.mean(lse_row - diag) + np.mean(lse_col - diag))
    return np.float32(loss)
